# revision 1
# baseline (speedup 1.0000x reference)
"""GAT (3-layer, DGL GATConv-style) on 8 Trainium2 NeuronCores.

Self-contained kernel: kernel(**inputs) takes the full unsharded inputs
(features [50000,256] f32, src/dst [800000] i32, per-layer W/al/ar/b),
distributes across 8 cores (dst-slab graph partition), runs one Bass
kernel launch per GAT layer, and returns the full [50000, 64] output.

Device-side design (per core, per layer):
  phase A: node table  [h@W | h@W@alm | h@W@arm]  (bf16 feat, fp16 el/er),
           written to DRAM in surrogate row order with 768B row stride.
  phase B: per-edge rows gathered with dma_gather (520B payload, int16
           indices into lo/hi table halves, 4 SWDGE queues);
           t = el[src]+er[dst] via PE (one-hot dst matrix @ er window + I@el);
           ex = max(exp(t), exp(0.2t))  == exp(leaky_relu(t, 0.2));
           weighted scatter-aggregation as PE matmul:
              psum[64dst, 260] += onehot_ed.T @ [feat*ex | ex]
           epilogue: rst = acc/den + (h+b); final layer adds relu + head-mean.
Graph structure (tile schedule, one-hot matrices, gather indices) is
precomputed on the host once and reused for all three layers.
"""

import sys

sys.path.insert(0, "/opt/trn_rl_repo")

import inspect
import textwrap

import numpy as np
import ml_dtypes

import concourse.bacc as bacc
import concourse.bass as bass
import concourse.mybir as mybir
import concourse.tile as tile
from concourse.masks import make_identity

F32 = mybir.dt.float32
F16 = mybir.dt.float16
BF16 = mybir.dt.bfloat16
I16 = mybir.dt.int16

BF = ml_dtypes.bfloat16

# --- patch dma_gather: drop the (transpose-only) elem_size%256 assert ---
_src = textwrap.dedent(inspect.getsource(bass.BassGpSimd.dma_gather))
_src = _src.replace("elem_size_bytes > 0 and elem_size_bytes % 256 == 0",
                    "elem_size_bytes > 0")
_src = _src.replace("def dma_gather(", "def _dma_gather_relaxed(", 1)
_ns = dict(bass.__dict__)
exec(compile(_src, "patched_dma_gather", "exec"), _ns)
bass.BassGpSimd.dma_gather_relaxed = _ns["_dma_gather_relaxed"]


class Cfg:
    def __init__(self, N, E, D, H, DH, n_cores, win=64, kblk=16, grp=4,
                 out_heads_mean=False):
        self.N = N
        self.E = E
        self.D = D
        self.H = H
        self.DH = DH
        self.C = n_cores
        self.WIN = win      # dst nodes per window (psum group)
        self.KBLK = kblk    # edge-tiles per compute block
        self.GRP = grp      # windows per gather group
        slab = -(-N // n_cores)
        slab = -(-slab // win) * win
        while (slab * n_cores) % 128:
            slab += win
        self.NSLAB = slab
        self.NPAD = slab * n_cores
        self.NW = slab // win
        assert self.NPAD % 128 == 0
        assert self.NSLAB % 128 == 0
        self.NT = self.NPAD // 128
        self.TS = self.NSLAB // 128          # own-slab tiles per core
        self.ROW = D + 2 * H                 # payload elements
        self.RSTRIDE = -(-(self.ROW * 2) // 256) * 128  # row stride elements
        self.HALF = min(32768, self.NPAD)
        self.HIBASE = self.NPAD - self.HALF
        self.out_heads_mean = out_heads_mean

    def surr(self, n):
        return (n % 128) * self.NT + n // 128


def plan_edges(cfg, src, dst):
    """Common tile schedule + per-core edge tensors.

    Tiles are grouped: per window, lo-half tiles then hi-half tiles
    (half = which table half the surrogate src row falls in); windows are
    grouped into gather groups of GRP windows.

    Returns dict:
      groups: list of group descriptors:
         dict(slots=[(w, half)...], lo=(s0,s1), hi=(s0,s1))
         (slot indices are global tile ids)
      wslots: per window list of global tile ids (in matmul order)
      T: total tiles
      eidx [C, 128, T*8] int16 (wrapped gather indices, per instruction layout)
      ohe [C, 128, T*WIN] bf16, ohd [C, 64, T*128] fp16
    """
    C, WIN, NW, NSLAB, GRP = cfg.C, cfg.WIN, cfg.NW, cfg.NSLAB, cfg.GRP
    core_of = dst // NSLAB
    dloc = dst % NSLAB
    win_of = dloc // WIN

    deg = np.zeros(cfg.NPAD, dtype=np.int64)
    np.add.at(deg, dst, 1)
    zdeg = deg == 0

    surr_src = cfg.surr(src.astype(np.int64))
    half_of = (surr_src >= cfg.HALF).astype(np.int64)  # 0 = lo, 1 = hi

    # counts per (core, window, half)
    cnt = np.zeros((C, NW, 2), dtype=np.int64)
    np.add.at(cnt, (core_of, win_of, half_of), 1)
    # fake edges (src=0 -> lo) for zero-degree dsts
    zz = np.nonzero(zdeg)[0]
    np.add.at(cnt, (zz // NSLAB, (zz % NSLAB) // WIN, np.zeros(len(zz), np.int64)), 1)

    t_lo = -(-cnt[:, :, 0].max(axis=0) // 128)
    t_hi = -(-cnt[:, :, 1].max(axis=0) // 128)
    # every window needs >= 1 tile total (fakes guarantee lo>=1 when needed)
    t_lo = np.maximum(t_lo, (t_lo + t_hi == 0).astype(np.int64))

    # global slot ids: grouped by (group, half, window)
    wslots = [[] for _ in range(NW)]
    hslots = {}          # (w, half) -> list of slot ids
    groups = []
    T = 0
    for g in range(-(-NW // GRP)):
        ws = list(range(g * GRP, min((g + 1) * GRP, NW)))
        slots = []
        lo0 = T
        for w in ws:
            hslots[(w, 0)] = list(range(T, T + int(t_lo[w])))
            wslots[w] += hslots[(w, 0)]
            slots += [(w, 0)] * int(t_lo[w])
            T += int(t_lo[w])
        lo1 = T
        for w in ws:
            hslots[(w, 1)] = list(range(T, T + int(t_hi[w])))
            wslots[w] += hslots[(w, 1)]
            slots += [(w, 1)] * int(t_hi[w])
            T += int(t_hi[w])
        hi1 = T
        groups.append(dict(slots=slots, lo=(lo0, lo1), hi=(lo1, hi1)))

    eidx = np.zeros((C, 128, T * 8), dtype=np.int16)
    ohe = np.zeros((C, 128, T * WIN), dtype=BF)
    ohd = np.zeros((C, 64, T * 128), dtype=np.float16)

    key = (core_of * NW + win_of) * 2 + half_of
    order = np.lexsort((dst, key))
    s_sorted = src[order]
    d_sorted = dst[order]
    cw = key[order]
    starts = np.searchsorted(cw, np.arange(C * NW * 2))
    ends = np.searchsorted(cw, np.arange(C * NW * 2) + 1)

    # wrap map: index i of a tile -> (row i%16, col i//16)
    wrap_r = np.arange(128) % 16
    wrap_c = np.arange(128) // 16

    for c in range(C):
        for w in range(NW):
            base_d = c * NSLAB + w * WIN
            for half in (0, 1):
                kk = (c * NW + w) * 2 + half
                i0, i1 = starts[kk], ends[kk]
                ss = list(s_sorted[i0:i1])
                dd = list((d_sorted[i0:i1] - base_d))
                if half == 0:
                    for dl in range(WIN):
                        if zdeg[base_d + dl]:
                            ss.append(0)
                            dd.append(dl)
                sl_ids = hslots[(w, half)]
                nslots = len(sl_ids) * 128
                assert len(ss) <= nslots, (c, w, half, len(ss), nslots)
                npad = nslots - len(ss)
                ss += [0] * npad
                dd += [-1] * npad
                ss = np.asarray(ss, dtype=np.int64)
                dd = np.asarray(dd, dtype=np.int64)
                rows = cfg.surr(ss)
                if half == 1:
                    rows = rows - cfg.HIBASE
                    rows = np.where(rows < 0, 0, rows)  # null pads -> hi row 0
                for j, t in enumerate(sl_ids):
                    rr = rows[j * 128:(j + 1) * 128]
                    ddj = dd[j * 128:(j + 1) * 128]
                    eidx[c, wrap_r, t * 8 + wrap_c] = rr.astype(np.int16)
                    p = np.nonzero(ddj >= 0)[0]
                    ohe[c, p, t * WIN + ddj[p]] = BF(1.0)
                    ohd[c, ddj[p], t * 128 + p] = np.float16(1.0)
    # replicate idx rows 0:16 across all 8 Q7 core groups
    for c in range(C):
        eidx[c] = np.tile(eidx[c, :16], (8, 1))
    return dict(groups=groups, wslots=wslots, T=T, eidx=eidx, ohe=ohe, ohd=ohd)


def pack_hT(cfg, h):
    NT = cfg.NT
    KC = cfg.D // 128
    out = np.zeros((128, NT * cfg.D), dtype=np.float16)
    for i in range(NT):
        for j in range(KC):
            out[:, i * cfg.D + j * 128:i * cfg.D + (j + 1) * 128] = (
                h[i * 128:(i + 1) * 128, j * 128:(j + 1) * 128].T.astype(np.float16)
            )
    return out


def make_wx(cfg, W, al, ar):
    H, DH = cfg.H, cfg.DH
    alm = np.zeros((cfg.D, H), dtype=np.float64)
    arm = np.zeros((cfg.D, H), dtype=np.float64)
    for h in range(H):
        alm[h * DH:(h + 1) * DH, h] = al[h]
        arm[h * DH:(h + 1) * DH, h] = ar[h]
    Wx = np.concatenate(
        [W.astype(np.float64), W.astype(np.float64) @ alm,
         W.astype(np.float64) @ arm], axis=1)
    return Wx.astype(np.float16)


def build_kernel(cfg, plan, final, dbg=False):
    N, D, H, ROW = cfg.NPAD, cfg.D, cfg.H, cfg.ROW
    WIN, KBLK = cfg.WIN, cfg.KBLK
    RST = cfg.RSTRIDE
    T = plan["T"]
    NT = cfg.NT
    KC = D // 128
    DEN = D + H
    OUTD = cfg.DH if (cfg.out_heads_mean and final) else D

    nc = bacc.Bacc("TRN2", target_bir_lowering=False, debug=False,
                   enable_asserts=False, num_devices=cfg.C, num_swdge_queues=4)

    hTp = nc.dram_tensor("hTp", [128, NT * D], F16, kind="ExternalInput")
    Wx = nc.dram_tensor("Wx", [D, ROW], F16, kind="ExternalInput")
    hb = nc.dram_tensor("hb", [cfg.NSLAB, D], F16, kind="ExternalInput")
    eidx = nc.dram_tensor("eidx", [128, T * 8], I16, kind="ExternalInput")
    ohe_d = nc.dram_tensor("ohe", [128, T * WIN], BF16, kind="ExternalInput")
    ohd_d = nc.dram_tensor("ohd", [64, T * 128], F16, kind="ExternalInput")
    out = nc.dram_tensor("out", [cfg.NSLAB, OUTD], F16, kind="ExternalOutput")
    table = nc.dram_tensor(
        "table", [N, RST], F16, kind="ExternalOutput" if dbg else "Internal")

    AB = 8
    assert NT % AB == 0

    with tile.TileContext(nc) as tc:
        with (
            tc.tile_pool(name="const", bufs=1) as cpool,
            tc.tile_pool(name="psT", bufs=2, space="PSUM") as psT,
            tc.tile_pool(name="psB", bufs=cfg.GRP, space="PSUM") as psB,
            tc.tile_pool(name="grow", bufs=3) as gpool,
            tc.tile_pool(name="oh", bufs=4) as opool,
            tc.tile_pool(name="exg", bufs=3) as xpool,
            tc.tile_pool(name="tt", bufs=4) as tpool,
            tc.tile_pool(name="epi", bufs=3) as epool,
        ):
            wx0 = cpool.tile([128, ROW], F16, tag="wx0")
            wx1 = cpool.tile([128, ROW], F16, tag="wx1")
            nc.sync.dma_start(out=wx0[:], in_=Wx[0:128, :])
            if KC > 1:
                nc.sync.dma_start(out=wx1[:], in_=Wx[128:256, :])
            eidx_t = cpool.tile([128, T * 8], I16, tag="eidx")
            nc.sync.dma_start(out=eidx_t[:], in_=eidx[:, :])
            ident = cpool.tile([128, 128], F16, tag="ident")
            make_identity(nc, ident[:])

            # --- phase A ---
            with (
                tc.tile_pool(name="hblk", bufs=3) as hpool,
                tc.tile_pool(name="rowblk", bufs=3) as rpool,
                tc.tile_pool(name="psA", bufs=2, space="PSUM") as psA,
            ):
                for blk in range(NT // AB):
                    hblk = hpool.tile([128, AB * D], F16)
                    nc.sync.dma_start(
                        out=hblk[:], in_=hTp[:, blk * AB * D:(blk + 1) * AB * D])
                    rowblk = rpool.tile([128, AB * ROW], F16)
                    for j in range(AB):
                        ps = psA.tile([128, ROW], F32)
                        for k in range(KC):
                            nc.tensor.matmul(
                                out=ps[:],
                                lhsT=hblk[:, j * D + k * 128:j * D + (k + 1) * 128],
                                rhs=(wx0 if k == 0 else wx1)[:],
                                start=(k == 0), stop=(k == KC - 1))
                        if j % 2 == 0:
                            nc.scalar.activation(
                                out=rowblk[:, j * ROW:j * ROW + D].bitcast(BF16),
                                in_=ps[:, 0:D],
                                func=mybir.ActivationFunctionType.Copy)
                        else:
                            nc.vector.tensor_copy(
                                out=rowblk[:, j * ROW:j * ROW + D].bitcast(BF16),
                                in_=ps[:, 0:D])
                        nc.vector.tensor_copy(
                            out=rowblk[:, j * ROW + D:(j + 1) * ROW],
                            in_=ps[:, D:ROW])
                    nc.sync.dma_start(
                        out=table[:, :].rearrange("(p i) c -> p i c", p=128)[
                            :, blk * AB:(blk + 1) * AB, 0:ROW],
                        in_=rowblk[:].rearrange("p (j c) -> p j c", c=ROW))

            # --- er windows: [64, NW*4] fp16, dynamic per-core offset ---
            # er of node (c*NSLAB + w*WIN + d) lives at table row
            # ((w%2)*64 + d)*NT + (c*TS + w//2), cols D+H..D+2H
            erwin = cpool.tile([64, cfg.NW * 4], F16, tag="erwin")
            pid = nc.sync.partition_id()
            er_off = pid * (cfg.TS * RST) + (D + H)
            ew3 = erwin[:].rearrange("p (a b h) -> p a b h", b=2, h=H)
            for bpar in range(2):
                er_ap = bass.AP(
                    table[:, :].tensor, er_off + bpar * (64 * NT * RST),
                    [[NT * RST, 64], [RST, cfg.NW // 2], [1, H]])
                nc.sync.dma_start(out=ew3[:, :, bpar, :], in_=er_ap)

            # --- phase B ---
            qn = [0]

            def win_of_slot(s):
                for w in range(cfg.NW):
                    if s in plan["wslots"][w]:
                        return w
                raise AssertionError

            slot_to_win = {}
            for w in range(cfg.NW):
                for s in plan["wslots"][w]:
                    slot_to_win[s] = w

            hb_r = hb[:, :].rearrange("(w d) c -> d w c", d=WIN)
            out_r = out[:, :].rearrange("(w d) c -> d w c", d=WIN)
            for g, grp in enumerate(plan["groups"]):
                s_begin = grp["lo"][0]
                s_end = grp["hi"][1]
                nslot = s_end - s_begin
                w_lo = g * cfg.GRP
                w_hi = min((g + 1) * cfg.GRP, cfg.NW)
                nwg = w_hi - w_lo
                hbg = epool.tile([WIN, cfg.GRP * D], F16, tag="hbg")
                nc.sync.dma_start(
                    out=hbg[:, 0:nwg * D].rearrange("d (w c) -> d w c", c=D),
                    in_=hb_r[:, w_lo:w_hi, :])
                og = epool.tile([WIN, cfg.GRP * OUTD], F16, tag="og")
                grow = gpool.tile([128, nslot * ROW], F16, tag="grow")
                CHUNK = 15  # tiles per gather call; small calls stay at
                            # pure desc-gen rate (no ring-reclaim stall)
                for half, (hh0, hh1) in (("lo", grp["lo"]), ("hi", grp["hi"])):
                    src_ap = (table[0:cfg.HALF, 0:ROW] if half == "lo"
                              else table[cfg.HIBASE:N, 0:ROW])
                    for h0 in range(hh0, hh1, CHUNK):
                        h1 = min(h0 + CHUNK, hh1)
                        ni = (h1 - h0) * 128
                        nc.gpsimd.dma_gather_relaxed(
                            out_ap=grow[:, (h0 - s_begin) * ROW:(h1 - s_begin) * ROW]
                            .rearrange("p (t e) -> p t e", e=ROW),
                            in_ap=src_ap,
                            idxs_ap=eidx_t[:, h0 * 8:h1 * 8],
                            num_idxs=ni, num_idxs_reg=ni,
                            elem_size=ROW, elem_step=RST,
                            single_packet=False, queue_num=qn[0] % 4)
                        qn[0] += 1

                accs = {}
                open_w = {}
                for b0 in range(s_begin, s_end, KBLK):
                    b1 = min(b0 + KBLK, s_end)
                    k = b1 - b0
                    ohe_b = opool.tile([128, KBLK * WIN], BF16, tag="ohe")
                    nc.scalar.dma_start(
                        out=ohe_b[:, 0:k * WIN],
                        in_=ohe_d[:, b0 * WIN:b1 * WIN])
                    ohd_b = opool.tile([64, KBLK * 128], F16, tag="ohd")
                    nc.scalar.dma_start(
                        out=ohd_b[:, 0:k * 128],
                        in_=ohd_d[:, b0 * 128:b1 * 128])
                    pst = psT.tile([128, KBLK * 4], F32)
                    if final:
                        # final layer: er-only matmul, el added on DVE —
                        # takes the PE attention matmuls off the gather chain
                        for j in range(k):
                            s = b0 + j
                            w = slot_to_win[s]
                            nc.tensor.matmul(
                                out=pst[:, j * 4:(j + 1) * 4],
                                lhsT=ohd_b[:, j * 128:(j + 1) * 128],
                                rhs=erwin[:, w * 4:(w + 1) * 4],
                                start=True, stop=True, skip_group_check=True)
                        grow_k0 = (grow[:, (b0 - s_begin) * ROW:(b1 - s_begin) * ROW]
                                   .rearrange("p (k c) -> p k c", c=ROW))
                        tsrc = tpool.tile([128, KBLK * 4], F16, tag="tt")
                        nc.vector.tensor_add(
                            out=tsrc[:, 0:k * 4].rearrange("p (k h) -> p k h", h=H),
                            in0=pst[:, 0:k * 4].rearrange("p (k h) -> p k h", h=H),
                            in1=grow_k0[:, :, D:D + H])
                        xin = tsrc
                    else:
                        for j in range(k):
                            s = b0 + j
                            w = slot_to_win[s]
                            nc.tensor.matmul(
                                out=pst[:, j * 4:(j + 1) * 4],
                                lhsT=ohd_b[:, j * 128:(j + 1) * 128],
                                rhs=erwin[:, w * 4:(w + 1) * 4],
                                start=True, stop=False, skip_group_check=True)
                            nc.tensor.matmul(
                                out=pst[:, j * 4:(j + 1) * 4],
                                lhsT=ident[:],
                                rhs=grow[:, (s - s_begin) * ROW + D:(s - s_begin) * ROW + D + H],
                                start=False, stop=True, skip_group_check=True)
                        xin = pst
                    xa = tpool.tile([128, KBLK * 4], BF16, tag="xa")
                    xb = tpool.tile([128, KBLK * 4], BF16, tag="xb")
                    nc.scalar.activation(
                        out=xa[:, 0:k * 4], in_=xin[:, 0:k * 4],
                        func=mybir.ActivationFunctionType.Exp)
                    nc.scalar.activation(
                        out=xb[:, 0:k * 4], in_=xin[:, 0:k * 4],
                        func=mybir.ActivationFunctionType.Exp, scale=0.2)
                    exg = xpool.tile([128, KBLK * DEN], BF16, tag="exg")
                    exg_k = exg[:, 0:k * DEN].rearrange("p (k c) -> p k c", c=DEN)
                    nc.vector.tensor_max(
                        out=exg_k[:, :, D:DEN],
                        in0=xa[:, 0:k * 4].rearrange("p (k h) -> p k h", h=H),
                        in1=xb[:, 0:k * 4].rearrange("p (k h) -> p k h", h=H))
                    grow_k = (grow[:, (b0 - s_begin) * ROW:(b1 - s_begin) * ROW]
                              .rearrange("p (k c) -> p k c", c=ROW))
                    feat_in = grow_k[:, :, 0:D].bitcast(BF16).rearrange(
                        "p k (h f) -> p k h f", f=cfg.DH)
                    ex_in = (exg_k[:, :, D:DEN]
                             .to_broadcast([128, k, H, cfg.DH]))
                    exg_out = exg_k[:, :, 0:D].rearrange(
                        "p k (h f) -> p k h f", f=cfg.DH)
                    nc.vector.tensor_mul(out=exg_out, in0=feat_in, in1=ex_in)

                    # scatter matmuls for this block
                    for j in range(k):
                        s = b0 + j
                        w = slot_to_win[s]
                        if w not in accs:
                            acc_w = psB.tile([WIN, DEN], F32, tag="acc")
                            accs[w] = acc_w
                            open_w[w] = 0
                        first = open_w[w] == 0
                        last = s == plan["wslots"][w][-1]
                        open_w[w] += 1
                        nc.tensor.matmul(
                            out=accs[w][:],
                            lhsT=ohe_b[:, j * WIN:(j + 1) * WIN],
                            rhs=exg[:, j * DEN:(j + 1) * DEN],
                            start=first, stop=last, skip_group_check=True)
                        if last:
                            acc = accs.pop(w)
                            wl = w - w_lo
                            rec = epool.tile([WIN, H], F32, tag="rec")
                            nc.vector.reciprocal(out=rec[:], in_=acc[:, D:DEN])
                            rst = epool.tile([WIN, D], F32, tag="rst")
                            rec_in = rec[:].to_broadcast([WIN, H, cfg.DH])
                            acc_in = acc[:, 0:D].rearrange(
                                "p (h f) -> p h f", f=cfg.DH)
                            rst_out = rst[:].rearrange(
                                "p (h f) -> p h f", f=cfg.DH)
                            nc.vector.tensor_mul(
                                out=rst_out, in0=acc_in, in1=rec_in)
                            if cfg.out_heads_mean and final:
                                nc.vector.tensor_add(
                                    out=rst[:], in0=rst[:],
                                    in1=hbg[:, wl * D:(wl + 1) * D])
                                nc.vector.tensor_relu(out=rst[:], in_=rst[:])
                                o = epool.tile([WIN, cfg.DH], F32, tag="o")
                                nc.vector.tensor_add(
                                    out=o[:], in0=rst[:, 0:cfg.DH],
                                    in1=rst[:, cfg.DH:2 * cfg.DH])
                                for hh in range(2, H):
                                    nc.vector.tensor_add(
                                        out=o[:], in0=o[:],
                                        in1=rst[:, hh * cfg.DH:(hh + 1) * cfg.DH])
                                nc.vector.tensor_scalar_mul(
                                    out=og[:, wl * OUTD:(wl + 1) * OUTD],
                                    in0=o[:], scalar1=1.0 / H)
                            else:
                                nc.vector.tensor_add(
                                    out=og[:, wl * OUTD:(wl + 1) * OUTD],
                                    in0=rst[:],
                                    in1=hbg[:, wl * D:(wl + 1) * D])
                nc.sync.dma_start(
                    out=out_r[:, w_lo:w_hi, :],
                    in_=og[:, 0:nwg * OUTD].rearrange("d (w c) -> d w c", c=OUTD))

    nc.compile()
    return nc


# ---------------------------------------------------------------------------
# kernel() entry point
# ---------------------------------------------------------------------------
_CACHE = {}

_N, _E, _D, _H, _DH = 50000, 800000, 256, 4, 64


def _get_built(src, dst):
    key = "built"
    if key in _CACHE:
        return _CACHE[key]
    cfg_mid = Cfg(_N, _E, _D, _H, _DH, n_cores=8, out_heads_mean=False)
    cfg_fin = Cfg(_N, _E, _D, _H, _DH, n_cores=8, out_heads_mean=True)
    plan = plan_edges(cfg_mid, src.astype(np.int64), dst.astype(np.int64))
    nc_mid = build_kernel(cfg_mid, plan, final=False)
    nc_fin = build_kernel(cfg_fin, plan, final=True)
    _CACHE[key] = (cfg_mid, cfg_fin, plan, nc_mid, nc_fin)
    return _CACHE[key]


def _make_in_maps(cfg, plan, h, W, al, ar, b):
    hTp = pack_hT(cfg, h)
    Wx = make_wx(cfg, W, al, ar)
    maps = []
    for c in range(cfg.C):
        sl = slice(c * cfg.NSLAB, (c + 1) * cfg.NSLAB)
        hb = (h[sl] + b[None, :]).astype(np.float16)
        maps.append(dict(hTp=hTp, Wx=Wx, hb=hb, eidx=plan["eidx"][c],
                         ohe=plan["ohe"][c], ohd=plan["ohd"][c]))
    return maps


def _assemble(cfg, results, outd):
    out = np.zeros((cfg.NPAD, outd), dtype=np.float32)
    for c in range(cfg.C):
        out[c * cfg.NSLAB:(c + 1) * cfg.NSLAB] = results[c]["out"].astype(np.float32)
    out[cfg.N:] = 0.0
    return out


def kernel(features, src, dst, W0, al0, ar0, b0, W1, al1, ar1, b1,
           W2, al2, ar2, b2, _collect_exec_ns=None):
    from concourse.bass_utils import run_bass_kernel_spmd

    features = np.asarray(features, dtype=np.float32)
    src = np.asarray(src)
    dst = np.asarray(dst)
    cfg_mid, cfg_fin, plan, nc_mid, nc_fin = _get_built(src, dst)

    layers = [
        (np.asarray(W0), np.asarray(al0), np.asarray(ar0), np.asarray(b0)),
        (np.asarray(W1), np.asarray(al1), np.asarray(ar1), np.asarray(b1)),
        (np.asarray(W2), np.asarray(al2), np.asarray(ar2), np.asarray(b2)),
    ]
    h = np.zeros((cfg_mid.NPAD, _D), dtype=np.float32)
    h[:_N] = features
    for li, (W, al, ar, b) in enumerate(layers):
        final = li == 2
        cfg = cfg_fin if final else cfg_mid
        nc = nc_fin if final else nc_mid
        maps = _make_in_maps(cfg, plan, h, W, al, ar, b)
        res = run_bass_kernel_spmd(
            nc, maps, list(range(8)),
            trace=_collect_exec_ns is not None)
        if _collect_exec_ns is not None:
            _collect_exec_ns.append(res.exec_time_ns)
        outd = _DH if final else _D
        h = _assemble(cfg, res.results, outd)
    return h[:_N].astype(np.float32)



# revision 4
# speedup vs baseline: 1.0731x; 1.0731x over previous
"""GAT (3-layer, DGL GATConv-style) on 8 Trainium2 NeuronCores.

Self-contained kernel: kernel(**inputs) takes the full unsharded inputs
(features [50000,256] f32, src/dst [800000] i32, per-layer W/al/ar/b),
distributes across 8 cores (dst-slab graph partition), runs one Bass
kernel launch per GAT layer, and returns the full [50000, 64] output.

Device-side design (per core, per layer):
  phase A: each core computes the node table for its OWN slab only:
           row = [feat bf16 x256 | el f16 x4 | er f16 x4] (528B in a
           768B-stride row), split into an A-part (first 3200 slab rows)
           and a B-part (last 3072).  Two 8-core DRAM AllGathers then
           replicate the full table to every core; gathers that only
           need A-rows start as soon as the first collective lands,
           overlapping the second.
  phase B: per-edge rows gathered with dma_gather (520B payload =
           bf16 feat + f16 el, int16 row indices into the A/B tables,
           4 SWDGE queues);
           t = el[src]+er[dst]  (er via one-hot PE matmul, el via DVE);
           ex = max(exp(t), exp(0.2t))  == exp(leaky_relu(t, 0.2));
           weighted scatter-aggregation as PE matmul:
              psum[64dst, 260] += onehot_ed(fp8).T @ [feat*ex | ex]
           epilogue: rst = acc/den + (h+b); final layer computes
           mean_h relu(rst) via scalar-engine relu with scale=1/4.
Graph structure (tile schedule, one-hot matrices, gather indices) is
precomputed on the host once and reused for all three layers.
"""

import sys

sys.path.insert(0, "/opt/trn_rl_repo")

import inspect
import textwrap

import numpy as np
import ml_dtypes

import concourse.bacc as bacc
import concourse.bass as bass
import concourse.mybir as mybir
import concourse.tile as tile

F32 = mybir.dt.float32
F16 = mybir.dt.float16
BF16 = mybir.dt.bfloat16
F8 = mybir.dt.float8e4
I16 = mybir.dt.int16
U8 = mybir.dt.uint8

BF = ml_dtypes.bfloat16
E4M3 = ml_dtypes.float8_e4m3

# --- patch dma_gather: drop the (transpose-only) elem_size%256 assert ---
_src = textwrap.dedent(inspect.getsource(bass.BassGpSimd.dma_gather))
_src = _src.replace("elem_size_bytes > 0 and elem_size_bytes % 256 == 0",
                    "elem_size_bytes > 0")
_src = _src.replace("def dma_gather(", "def _dma_gather_relaxed(", 1)
_ns = dict(bass.__dict__)
exec(compile(_src, "patched_dma_gather", "exec"), _ns)
bass.BassGpSimd.dma_gather_relaxed = _ns["_dma_gather_relaxed"]


class Cfg:
    def __init__(self, N, E, D, H, DH, n_cores, win=64, kblk=16, grp=4,
                 out_heads_mean=False):
        self.N = N
        self.E = E
        self.D = D
        self.H = H
        self.DH = DH
        self.C = n_cores
        self.WIN = win      # dst nodes per window (psum group)
        self.KBLK = kblk    # edge-tiles per compute block
        self.GRP = grp      # windows per gather group
        slab = -(-N // n_cores)
        slab = -(-slab // win) * win
        while (slab * n_cores) % 128:
            slab += win
        self.NSLAB = slab
        self.NPAD = slab * n_cores
        self.NW = slab // win
        assert self.NPAD % 128 == 0
        assert self.NSLAB % 128 == 0
        self.NT = self.NPAD // 128
        self.TS = self.NSLAB // 128          # own-slab tiles per core
        # A/B slab halves (tile-aligned); both gathered tables < 32768 rows
        self.TSA = (self.TS + 1) // 2        # 25 tiles
        self.TSB = self.TS - self.TSA        # 24 tiles
        self.SA = self.TSA * 128             # 3200 rows per slab in A
        self.SB = self.TSB * 128             # 3072 rows per slab in B
        assert self.SA * n_cores <= 32768 and self.SB * n_cores <= 32768
        self.WXC = D + 2 * H                 # Wx columns: feat | el | er
        self.ROWB = 2 * D + 2 * H            # gathered payload bytes
        self.RSTB = 768                      # table row stride bytes
        self.out_heads_mean = out_heads_mean


def plan_edges(cfg, src, dst):
    """Common tile schedule + per-core edge tensors.

    Tiles are grouped: per window, lo(A)-half tiles then hi(B)-half
    tiles (half = which slab-half the src node's table row lives in);
    windows are grouped into gather groups of GRP windows.  Edges
    within a (core, window, half) segment are sorted by src row for
    gather locality.
    """
    C, WIN, NW, NSLAB, GRP = cfg.C, cfg.WIN, cfg.NW, cfg.NSLAB, cfg.GRP
    core_of = dst // NSLAB
    dloc = dst % NSLAB
    win_of = dloc // WIN

    deg = np.zeros(cfg.NPAD, dtype=np.int64)
    np.add.at(deg, dst, 1)
    zdeg = deg == 0

    sloc = src % NSLAB
    half_of = (sloc >= cfg.SA).astype(np.int64)  # 0 = A, 1 = B
    row_of = np.where(half_of == 0,
                      (src // NSLAB) * cfg.SA + sloc,
                      (src // NSLAB) * cfg.SB + (sloc - cfg.SA))

    # counts per (core, window, half)
    cnt = np.zeros((C, NW, 2), dtype=np.int64)
    np.add.at(cnt, (core_of, win_of, half_of), 1)
    # fake edges (src row 0 -> A) for zero-degree dsts
    zz = np.nonzero(zdeg)[0]
    np.add.at(cnt, (zz // NSLAB, (zz % NSLAB) // WIN, np.zeros(len(zz), np.int64)), 1)

    t_lo = -(-cnt[:, :, 0].max(axis=0) // 128)
    t_hi = -(-cnt[:, :, 1].max(axis=0) // 128)
    # every window needs >= 1 tile total (fakes guarantee lo>=1 when needed)
    t_lo = np.maximum(t_lo, (t_lo + t_hi == 0).astype(np.int64))

    # global slot ids: grouped by (group, half, window)
    wslots = [[] for _ in range(NW)]
    hslots = {}          # (w, half) -> list of slot ids
    groups = []
    T = 0
    for g in range(-(-NW // GRP)):
        ws = list(range(g * GRP, min((g + 1) * GRP, NW)))
        slots = []
        lo0 = T
        for w in ws:
            hslots[(w, 0)] = list(range(T, T + int(t_lo[w])))
            wslots[w] += hslots[(w, 0)]
            slots += [(w, 0)] * int(t_lo[w])
            T += int(t_lo[w])
        lo1 = T
        for w in ws:
            hslots[(w, 1)] = list(range(T, T + int(t_hi[w])))
            wslots[w] += hslots[(w, 1)]
            slots += [(w, 1)] * int(t_hi[w])
            T += int(t_hi[w])
        hi1 = T
        groups.append(dict(slots=slots, lo=(lo0, lo1), hi=(lo1, hi1)))

    eidx = np.zeros((C, 128, T * 8), dtype=np.int16)
    ohe = np.zeros((C, 128, T * WIN), dtype=E4M3)
    ohd = np.zeros((C, 64, T * 128), dtype=np.float16)

    key = (core_of * NW + win_of) * 2 + half_of
    order = np.lexsort((row_of, key))        # sort by src row within segments
    r_sorted = row_of[order]
    d_sorted = dst[order]
    cw = key[order]
    starts = np.searchsorted(cw, np.arange(C * NW * 2))
    ends = np.searchsorted(cw, np.arange(C * NW * 2) + 1)

    # wrap map: index i of a tile -> (row i%16, col i//16)
    wrap_r = np.arange(128) % 16
    wrap_c = np.arange(128) // 16

    for c in range(C):
        for w in range(NW):
            base_d = c * NSLAB + w * WIN
            for half in (0, 1):
                kk = (c * NW + w) * 2 + half
                i0, i1 = starts[kk], ends[kk]
                rr_ = list(r_sorted[i0:i1])
                dd = list((d_sorted[i0:i1] - base_d))
                if half == 0:
                    for dl in range(WIN):
                        if zdeg[base_d + dl]:
                            rr_.append(0)
                            dd.append(dl)
                sl_ids = hslots[(w, half)]
                nslots = len(sl_ids) * 128
                assert len(rr_) <= nslots, (c, w, half, len(rr_), nslots)
                npad = nslots - len(rr_)
                rr_ += [0] * npad
                dd += [-1] * npad
                rows = np.asarray(rr_, dtype=np.int64)
                dd = np.asarray(dd, dtype=np.int64)
                for j, t in enumerate(sl_ids):
                    rr = rows[j * 128:(j + 1) * 128]
                    ddj = dd[j * 128:(j + 1) * 128]
                    eidx[c, wrap_r, t * 8 + wrap_c] = rr.astype(np.int16)
                    p = np.nonzero(ddj >= 0)[0]
                    ohe[c, p, t * WIN + ddj[p]] = E4M3(1.0)
                    ohd[c, ddj[p], t * 128 + p] = np.float16(1.0)
    # replicate idx rows 0:16 across all 8 Q7 core groups
    for c in range(C):
        eidx[c] = np.tile(eidx[c, :16], (8, 1))
    return dict(groups=groups, wslots=wslots, T=T, eidx=eidx, ohe=ohe, ohd=ohd)


def pack_hT(cfg, h):
    """[NPAD, D] f32 -> [128, NT*D] f16 with cols (tile, kchunk, row):
    out[p, i*D + k*128 + q] = h[i*128 + q, k*128 + p]."""
    NT, D = cfg.NT, cfg.D
    KC = D // 128
    h4 = h.reshape(NT, 128, KC, 128).astype(np.float16)
    return np.ascontiguousarray(h4.transpose(3, 0, 2, 1).reshape(128, NT * D))


def make_wx(cfg, W, al, ar):
    H, DH = cfg.H, cfg.DH
    alm = np.zeros((cfg.D, H), dtype=np.float64)
    arm = np.zeros((cfg.D, H), dtype=np.float64)
    for h in range(H):
        alm[h * DH:(h + 1) * DH, h] = al[h]
        arm[h * DH:(h + 1) * DH, h] = ar[h]
    Wx = np.concatenate(
        [W.astype(np.float64), W.astype(np.float64) @ alm,
         W.astype(np.float64) @ arm], axis=1)
    return Wx.astype(np.float16)


def build_kernel(cfg, plan, final):
    D, H = cfg.D, cfg.H
    WIN, KBLK = cfg.WIN, cfg.KBLK
    ROWB, RSTB, WXC = cfg.ROWB, cfg.RSTB, cfg.WXC
    T = plan["T"]
    KC = D // 128
    DEN = D + H                            # 260 scatter columns
    FB = 2 * D                             # feat bytes in a row
    OUTD = cfg.DH if (cfg.out_heads_mean and final) else D

    nc = bacc.Bacc("TRN2", target_bir_lowering=False, debug=False,
                   enable_asserts=False, num_devices=cfg.C, num_swdge_queues=4)

    hTs = nc.dram_tensor("hTs", [128, cfg.TS * D], F16, kind="ExternalInput")
    Wx = nc.dram_tensor("Wx", [D, WXC], F16, kind="ExternalInput")
    hb = nc.dram_tensor("hb", [cfg.NSLAB, D], F16, kind="ExternalInput")
    eidx = nc.dram_tensor("eidx", [128, T * 8], I16, kind="ExternalInput")
    ohe_d = nc.dram_tensor("ohe", [128, T * WIN], F8, kind="ExternalInput")
    ohd_d = nc.dram_tensor("ohd", [64, T * 128], F16, kind="ExternalInput")
    out = nc.dram_tensor("out", [cfg.NSLAB, OUTD], F16, kind="ExternalOutput")
    tabSA = nc.dram_tensor("tabSA", [cfg.SA, RSTB], U8, kind="Internal")
    tabSB = nc.dram_tensor("tabSB", [cfg.SB, RSTB], U8, kind="Internal")
    tabA = nc.dram_tensor("tabA", [cfg.SA * cfg.C, RSTB], U8, kind="Internal",
                          addr_space="Shared")
    tabB = nc.dram_tensor("tabB", [cfg.SB * cfg.C, RSTB], U8, kind="Internal",
                          addr_space="Shared")

    with tile.TileContext(nc) as tc:
        with (
            tc.tile_pool(name="const", bufs=1) as cpool,
            tc.tile_pool(name="psT", bufs=2, space="PSUM") as psT,
            tc.tile_pool(name="psB", bufs=cfg.GRP, space="PSUM") as psB,
            tc.tile_pool(name="grow", bufs=3) as gpool,
            tc.tile_pool(name="oh", bufs=4) as opool,
            tc.tile_pool(name="exg", bufs=3) as xpool,
            tc.tile_pool(name="tt", bufs=4) as tpool,
            tc.tile_pool(name="epi", bufs=3) as epool,
        ):
            wx0 = cpool.tile([128, WXC], F16, tag="wx0")
            wx1 = cpool.tile([128, WXC], F16, tag="wx1")
            nc.sync.dma_start(out=wx0[:], in_=Wx[0:128, :])
            nc.sync.dma_start(out=wx1[:], in_=Wx[128:256, :])
            eidx_t = cpool.tile([128, T * 8], I16, tag="eidx")
            nc.sync.dma_start(out=eidx_t[:], in_=eidx[:, :])

            # --- phase A: own slab, A-half then B-half ---
            erwin = cpool.tile([64, cfg.NW * H], F16, tag="erwin")
            with (
                tc.tile_pool(name="hblk", bufs=3) as hpool,
                tc.tile_pool(name="fblk", bufs=3) as fpool,
                tc.tile_pool(name="psA", bufs=2, space="PSUM") as psA,
            ):
                def phase_a(tabS, t0, nt, AB):
                    tabS_r = tabS[:, :].rearrange("(i p) c -> p i c", p=128)
                    for blk in range(nt // AB):
                        hblk = hpool.tile([128, 8 * D], F16)
                        i0 = (t0 + blk * AB) * D
                        nc.sync.dma_start(
                            out=hblk[:, 0:AB * D], in_=hTs[:, i0:i0 + AB * D])
                        fblk = fpool.tile([128, 8 * D], BF16, tag="fblk")
                        eblk = fpool.tile([128, 8 * 2 * H], F16, tag="eblk")
                        for j in range(AB):
                            ps = psA.tile([128, WXC], F32)
                            for k in range(KC):
                                nc.tensor.matmul(
                                    out=ps[:],
                                    lhsT=hblk[:, j * D + k * 128:j * D + (k + 1) * 128],
                                    rhs=(wx0 if k == 0 else wx1)[:],
                                    start=(k == 0), stop=(k == KC - 1))
                            nc.vector.tensor_copy(
                                out=fblk[:, j * D:(j + 1) * D],
                                in_=ps[:, 0:D])
                            nc.scalar.activation(
                                out=eblk[:, j * 2 * H:(j + 1) * 2 * H],
                                in_=ps[:, D:D + 2 * H],
                                func=mybir.ActivationFunctionType.Copy)
                        nc.sync.dma_start(
                            out=tabS_r[:, blk * AB:(blk + 1) * AB, 0:FB]
                            .bitcast(BF16),
                            in_=fblk[:, 0:AB * D].rearrange("p (j c) -> p j c", c=D))
                        nc.sync.dma_start(
                            out=tabS_r[:, blk * AB:(blk + 1) * AB,
                                       FB:FB + 4 * H].bitcast(F16),
                            in_=eblk[:, 0:AB * 2 * H].rearrange(
                                "p (j c) -> p j c", c=2 * H))

                phase_a(tabSA, 0, cfg.TSA, 5)
                # er windows for the A-half (windows 0 .. SA/WIN)
                nwa = cfg.SA // WIN
                er_apA = (tabSA[:, :].rearrange("(w d) c -> d w c", d=WIN)
                          [:, :, FB + 2 * H:FB + 4 * H].bitcast(F16))
                nc.sync.dma_start(
                    out=erwin[:, 0:nwa * H].rearrange("p (w h) -> p w h", h=H),
                    in_=er_apA)
                nc.gpsimd.collective_compute(
                    kind="AllGather", op=mybir.AluOpType.bypass,
                    replica_groups=[list(range(cfg.C))],
                    ins=[tabSA[:, :]], outs=[tabA[:, :]])
                phase_a(tabSB, cfg.TSA, cfg.TSB, 6)
                er_apB = (tabSB[:, :].rearrange("(w d) c -> d w c", d=WIN)
                          [:, :, FB + 2 * H:FB + 4 * H].bitcast(F16))
                nc.sync.dma_start(
                    out=erwin[:, nwa * H:].rearrange("p (w h) -> p w h", h=H),
                    in_=er_apB)
                nc.gpsimd.collective_compute(
                    kind="AllGather", op=mybir.AluOpType.bypass,
                    replica_groups=[list(range(cfg.C))],
                    ins=[tabSB[:, :]], outs=[tabB[:, :]])

            # --- phase B ---
            qn = [0]
            slot_to_win = {}
            for w in range(cfg.NW):
                for s in plan["wslots"][w]:
                    slot_to_win[s] = w

            hb_r = hb[:, :].rearrange("(w d) c -> d w c", d=WIN)
            out_r = out[:, :].rearrange("(w d) c -> d w c", d=WIN)
            for g, grp in enumerate(plan["groups"]):
                s_begin = grp["lo"][0]
                s_end = grp["hi"][1]
                nslot = s_end - s_begin
                w_lo = g * cfg.GRP
                w_hi = min((g + 1) * cfg.GRP, cfg.NW)
                nwg = w_hi - w_lo
                hbg = epool.tile([WIN, cfg.GRP * D], F16, tag="hbg")
                nc.sync.dma_start(
                    out=hbg[:, 0:nwg * D].rearrange("d (w c) -> d w c", c=D),
                    in_=hb_r[:, w_lo:w_hi, :])
                og = epool.tile([WIN, cfg.GRP * OUTD], F16, tag="og")
                grow = gpool.tile([128, nslot * ROWB], U8, tag="grow")
                CHUNK = 15  # tiles per gather call; small calls stay at
                            # pure desc-gen rate (no ring-reclaim stall)
                for half, (hh0, hh1) in (("lo", grp["lo"]), ("hi", grp["hi"])):
                    src_ap = (tabA[:, 0:ROWB] if half == "lo"
                              else tabB[:, 0:ROWB])
                    for h0 in range(hh0, hh1, CHUNK):
                        h1 = min(h0 + CHUNK, hh1)
                        ni = (h1 - h0) * 128
                        nc.gpsimd.dma_gather_relaxed(
                            out_ap=grow[:, (h0 - s_begin) * ROWB:(h1 - s_begin) * ROWB]
                            .rearrange("p (t e) -> p t e", e=ROWB),
                            in_ap=src_ap,
                            idxs_ap=eidx_t[:, h0 * 8:h1 * 8],
                            num_idxs=ni, num_idxs_reg=ni,
                            elem_size=ROWB, elem_step=RSTB,
                            single_packet=False, queue_num=qn[0] % 4)
                        qn[0] += 1

                accs = {}
                open_w = {}
                for b0 in range(s_begin, s_end, KBLK):
                    b1 = min(b0 + KBLK, s_end)
                    k = b1 - b0
                    ohe_b = opool.tile([128, KBLK * WIN], F8, tag="ohe")
                    nc.scalar.dma_start(
                        out=ohe_b[:, 0:k * WIN],
                        in_=ohe_d[:, b0 * WIN:b1 * WIN])
                    ohd_b = opool.tile([64, KBLK * 128], F16, tag="ohd")
                    nc.scalar.dma_start(
                        out=ohd_b[:, 0:k * 128],
                        in_=ohd_d[:, b0 * 128:b1 * 128])
                    pst = psT.tile([128, KBLK * H], F32)
                    for j in range(k):
                        s = b0 + j
                        w = slot_to_win[s]
                        nc.tensor.matmul(
                            out=pst[:, j * H:(j + 1) * H],
                            lhsT=ohd_b[:, j * 128:(j + 1) * 128],
                            rhs=erwin[:, w * H:(w + 1) * H],
                            start=True, stop=True, skip_group_check=True)
                    grow_k = (grow[:, (b0 - s_begin) * ROWB:(b1 - s_begin) * ROWB]
                              .rearrange("p (t e) -> p t e", e=ROWB))
                    tsrc = tpool.tile([128, KBLK * H], BF16, tag="tt")
                    nc.vector.tensor_add(
                        out=tsrc[:, 0:k * H].rearrange("p (k h) -> p k h", h=H),
                        in0=pst[:, 0:k * H].rearrange("p (k h) -> p k h", h=H),
                        in1=grow_k[:, :, FB:FB + 2 * H].bitcast(F16))
                    xa = tpool.tile([128, KBLK * H], BF16, tag="xa")
                    xb = tpool.tile([128, KBLK * H], BF16, tag="xb")
                    nc.scalar.activation(
                        out=xa[:, 0:k * H], in_=tsrc[:, 0:k * H],
                        func=mybir.ActivationFunctionType.Exp)
                    nc.scalar.activation(
                        out=xb[:, 0:k * H], in_=tsrc[:, 0:k * H],
                        func=mybir.ActivationFunctionType.Exp, scale=0.2)
                    exg = xpool.tile([128, KBLK * DEN], BF16, tag="exg")
                    exg_k = exg[:, 0:k * DEN].rearrange("p (k c) -> p k c", c=DEN)
                    nc.vector.tensor_max(
                        out=exg_k[:, :, D:DEN],
                        in0=xa[:, 0:k * H].rearrange("p (k h) -> p k h", h=H),
                        in1=xb[:, 0:k * H].rearrange("p (k h) -> p k h", h=H))
                    feat_in = grow_k[:, :, 0:FB].bitcast(BF16).rearrange(
                        "p k (h f) -> p k h f", f=cfg.DH)
                    ex_in = (exg_k[:, :, D:DEN]
                             .to_broadcast([128, k, H, cfg.DH]))
                    exg_out = exg_k[:, :, 0:D].rearrange(
                        "p k (h f) -> p k h f", f=cfg.DH)
                    nc.vector.tensor_mul(out=exg_out, in0=feat_in, in1=ex_in)

                    # scatter matmuls for this block
                    for j in range(k):
                        s = b0 + j
                        w = slot_to_win[s]
                        if w not in accs:
                            acc_w = psB.tile([WIN, DEN], F32, tag="acc")
                            accs[w] = acc_w
                            open_w[w] = 0
                        first = open_w[w] == 0
                        last = s == plan["wslots"][w][-1]
                        open_w[w] += 1
                        nc.tensor.matmul(
                            out=accs[w][:],
                            lhsT=ohe_b[:, j * WIN:(j + 1) * WIN],
                            rhs=exg[:, j * DEN:(j + 1) * DEN],
                            start=first, stop=last, skip_group_check=True)
                        if last:
                            acc = accs.pop(w)
                            wl = w - w_lo
                            rec = epool.tile([WIN, H], F32, tag="rec")
                            nc.vector.reciprocal(out=rec[:], in_=acc[:, D:DEN])
                            rst = epool.tile([WIN, D], F32, tag="rst")
                            rec_in = rec[:].to_broadcast([WIN, H, cfg.DH])
                            acc_in = acc[:, 0:D].rearrange(
                                "p (h f) -> p h f", f=cfg.DH)
                            rst_out = rst[:].rearrange(
                                "p (h f) -> p h f", f=cfg.DH)
                            nc.vector.tensor_mul(
                                out=rst_out, in0=acc_in, in1=rec_in)
                            if cfg.out_heads_mean and final:
                                nc.vector.tensor_add(
                                    out=rst[:], in0=rst[:],
                                    in1=hbg[:, wl * D:(wl + 1) * D])
                                rq = epool.tile([WIN, D], BF16, tag="rq")
                                nc.scalar.activation(
                                    out=rq[:], in_=rst[:],
                                    func=mybir.ActivationFunctionType.Relu,
                                    scale=0.25)
                                o1 = epool.tile([WIN, cfg.DH], F32, tag="o1")
                                nc.vector.tensor_add(
                                    out=o1[:], in0=rq[:, 0:cfg.DH],
                                    in1=rq[:, cfg.DH:2 * cfg.DH])
                                o2 = epool.tile([WIN, cfg.DH], F32, tag="o2")
                                nc.vector.tensor_add(
                                    out=o2[:], in0=rq[:, 2 * cfg.DH:3 * cfg.DH],
                                    in1=rq[:, 3 * cfg.DH:4 * cfg.DH])
                                nc.vector.tensor_add(
                                    out=og[:, wl * OUTD:(wl + 1) * OUTD],
                                    in0=o1[:], in1=o2[:])
                            else:
                                nc.vector.tensor_add(
                                    out=og[:, wl * OUTD:(wl + 1) * OUTD],
                                    in0=rst[:],
                                    in1=hbg[:, wl * D:(wl + 1) * D])
                nc.sync.dma_start(
                    out=out_r[:, w_lo:w_hi, :],
                    in_=og[:, 0:nwg * OUTD].rearrange("d (w c) -> d w c", c=OUTD))

    nc.compile()
    return nc


# ---------------------------------------------------------------------------
# kernel() entry point
# ---------------------------------------------------------------------------
_CACHE = {}

_N, _E, _D, _H, _DH = 50000, 800000, 256, 4, 64


def _get_built(src, dst):
    key = "built"
    if key in _CACHE:
        return _CACHE[key]
    cfg_mid = Cfg(_N, _E, _D, _H, _DH, n_cores=8, out_heads_mean=False)
    cfg_fin = Cfg(_N, _E, _D, _H, _DH, n_cores=8, out_heads_mean=True)
    plan = plan_edges(cfg_mid, src.astype(np.int64), dst.astype(np.int64))
    nc_mid = build_kernel(cfg_mid, plan, final=False)
    nc_fin = build_kernel(cfg_fin, plan, final=True)
    _CACHE[key] = (cfg_mid, cfg_fin, plan, nc_mid, nc_fin)
    return _CACHE[key]


def _make_in_maps(cfg, plan, h, W, al, ar, b):
    hTp = pack_hT(cfg, h)
    Wx = make_wx(cfg, W, al, ar)
    maps = []
    for c in range(cfg.C):
        sl = slice(c * cfg.NSLAB, (c + 1) * cfg.NSLAB)
        hb = (h[sl] + b[None, :]).astype(np.float16)
        hTs = hTp[:, c * cfg.TS * cfg.D:(c + 1) * cfg.TS * cfg.D]
        maps.append(dict(hTs=hTs, Wx=Wx, hb=hb, eidx=plan["eidx"][c],
                         ohe=plan["ohe"][c], ohd=plan["ohd"][c]))
    return maps


def _assemble(cfg, results, outd):
    out = np.zeros((cfg.NPAD, outd), dtype=np.float32)
    for c in range(cfg.C):
        out[c * cfg.NSLAB:(c + 1) * cfg.NSLAB] = results[c]["out"].astype(np.float32)
    out[cfg.N:] = 0.0
    return out


def kernel(features, src, dst, W0, al0, ar0, b0, W1, al1, ar1, b1,
           W2, al2, ar2, b2, _collect_exec_ns=None):
    from concourse.bass_utils import run_bass_kernel_spmd

    features = np.asarray(features, dtype=np.float32)
    src = np.asarray(src)
    dst = np.asarray(dst)
    cfg_mid, cfg_fin, plan, nc_mid, nc_fin = _get_built(src, dst)

    layers = [
        (np.asarray(W0), np.asarray(al0), np.asarray(ar0), np.asarray(b0)),
        (np.asarray(W1), np.asarray(al1), np.asarray(ar1), np.asarray(b1)),
        (np.asarray(W2), np.asarray(al2), np.asarray(ar2), np.asarray(b2)),
    ]
    h = np.zeros((cfg_mid.NPAD, _D), dtype=np.float32)
    h[:_N] = features
    for li, (W, al, ar, b) in enumerate(layers):
        final = li == 2
        cfg = cfg_fin if final else cfg_mid
        nc = nc_fin if final else nc_mid
        maps = _make_in_maps(cfg, plan, h, W, al, ar, b)
        res = run_bass_kernel_spmd(
            nc, maps, list(range(8)),
            trace=_collect_exec_ns is not None)
        if _collect_exec_ns is not None:
            _collect_exec_ns.append(res.exec_time_ns)
        outd = _DH if final else _D
        h = _assemble(cfg, res.results, outd)
    return h[:_N].astype(np.float32)
